# revision 10
# baseline (speedup 1.0000x reference)
"""Inner-policy-sharded Trainium2 kernel for DecoupledDynamicsModel (MoE).

Model: B=8192 rows; each row selects one of P=8 outer policies via
policy_indices; the selected policy runs 8 inner MLPs (72 -> 512 -> 512 -> 64)
on (latent chunk, action) and the 8 inner outputs concatenate to 512 dims.

Sharding: by INNER policy. Core i computes inner MLP i for every row, using
the row's outer-policy weight set W*[outer, i]. Rows are sorted by outer
policy on the host so tokens form 8 contiguous groups; within a group the
weights are stationary. Perfect load balance, no capacity padding.

Numerics / engine plan (vs the f32r baseline):
- Layer 1 (K=73 incl. a ones-row carrying b1): bf16 feature-major matmul,
  4 m-chunks. PSUM holds x@W1 + b1 directly.
- Layer 1 eviction produces the layer-2 moving operand as an fp8 PAIR:
    H = fp8(relu(ps))            on ACT (activation Relu -> float8e4)
    L = fp8(relu(ps) - H)        on DVE (one fused scalar_tensor_tensor)
  so h1 ~= H + L to ~0.1% while both factors are fp8.
- Layer 2: fp8e4 DoubleRow matmuls (0.5 cycles/row): 3 compensated terms
  H@Wq + L@Wq + H@Wr where Wq = fp8(256*W2), Wr = fp8(256*W2 - Wq) are
  split on the host. All three share one PSUM scale (256). 6 DoubleRow
  matmuls per m-chunk replace 16 full-rate k-chunks.
- Layer 2 eviction h2s = max(ps + 256*b2, 0) -> bf16 on Pool (gpsimd).
  h2s = 256*h2; the 1/256 is folded into W3 on the host.
- Layer 3: token-major bf16 matmul: stationary = h2s chunk [128, 128 tok],
  moving = W3/256 [128, 64] -> PSUM [tokens, 64]; 2 cycles/token instead
  of 4 (no half-empty 64-wide output partitions). b3 is added on the host.
- y eviction on ACT (Copy), DMA out token-major per tile slot.

PE work: 4 (L1) + 12 (L2) + 2 (L3) = 18 cycles/token vs 24 for the f32r
baseline. Emission is software-pipelined [L1(t), L2(t-1), L3(t-2)] so the
PE never waits on same-tile evictions.
"""

import sys

sys.path.insert(0, "/opt/trn_rl_repo")

import numpy as np
import ml_dtypes

import concourse.bass as bass
from concourse import bacc
import concourse.mybir as mybir
import concourse.tile as tile
from concourse.bass import ts
from concourse.bass_utils import run_bass_kernel_spmd

P = 8          # outer policies == n_cores == inner MLPs per policy
Z = 64         # per-policy latent dim
D = P * Z      # 512
A = 8          # action dim
IN = Z + A     # 72, MLP input dim
INB = IN + 1   # 73: ones-row carries b1 into the L1 matmul
H = 512        # hidden dim
NCORES = 8

F32 = mybir.dt.float32
BF16 = mybir.dt.bfloat16
F8 = mybir.dt.float8e4
RELU = mybir.ActivationFunctionType.Relu
COPY = mybir.ActivationFunctionType.Copy
ADD = mybir.AluOpType.add
MAX = mybir.AluOpType.max
SUB = mybir.AluOpType.subtract
DR = mybir.MatmulPerfMode.DoubleRow

NP_BF16 = ml_dtypes.bfloat16
NP_F8 = ml_dtypes.float8_e4m3

WSCALE = 256.0  # host scale on W2 before fp8 split; folded back via b2/W3

TRACE = False
REPEAT = 1
LAST_RESULT = None


def _group_tiles(counts):
    """Token tiles for the sorted stream: each tile stays inside one outer-
    policy group; sizes <=512, multiples of 4 (group padding)."""
    tiles = []
    off = 0
    for g, n in enumerate(counts):
        r = n
        if g == 0 and r > 768:
            # small leading tile so compute starts on fewer loaded bytes
            tiles.append((g, off, 256))
            off += 256
            r -= 256
        while r > 0:
            if r > 768:
                t = 512
            elif r > 512:
                t = r - 256
            else:
                t = r
            tiles.append((g, off, t))
            off += t
            r -= t
    return tiles


def _build_program(counts, B, repeat=1):
    tiles = _group_tiles(counts)
    nt = len(tiles)
    nc = bacc.Bacc()

    xTd = nc.declare_dram_parameter("xT", [INB, B], BF16, isOutput=False)
    w1d = nc.declare_dram_parameter("w1", [P, INB, H], BF16, isOutput=False)
    wqrd = nc.declare_dram_parameter("wqr", [P, 128, 8, H], F8, isOutput=False)
    w3d = nc.declare_dram_parameter("w3", [128, P, 4, Z], BF16, isOutput=False)
    b2d = nc.declare_dram_parameter("b2", [128, 4 * P], F32, isOutput=False)
    yd = nc.declare_dram_parameter("y", [nt, 128, 4, Z], F32, isOutput=True)

    with tile.TileContext(nc) as tc:
        with (
            tc.tile_pool(name="w1p", bufs=P) as w1pool,
            tc.tile_pool(name="wqrp", bufs=P) as wqrpool,
            tc.tile_pool(name="cst", bufs=1) as cstpool,
            tc.tile_pool(name="xs", bufs=1) as xpool,
            tc.tile_pool(name="gs", bufs=2) as gpool,
            tc.tile_pool(name="hl", bufs=2) as hlpool,
            tc.tile_pool(name="h2", bufs=3) as h2pool,
            tc.tile_pool(name="ys", bufs=2) as ypool,
            tc.tile_pool(name="ps1", bufs=2, space="PSUM") as ps1pool,
            tc.tile_pool(name="ps2", bufs=3, space="PSUM") as ps2pool,
            tc.tile_pool(name="ps3", bufs=1, space="PSUM") as ps3pool,
        ):
            for _rep in range(repeat):
                xt = xpool.tile([INB, B], BF16, tag="x")
                w3t = cstpool.tile([128, P, 4, Z], BF16, tag="w3")
                b2t = cstpool.tile([128, 4 * P], F32, tag="b2")
                w1ts, wqrts = [], []
                for _g in range(P):
                    w1_t = w1pool.tile([INB, H], BF16, tag="w1")
                    wqr_t = wqrpool.tile([128, 8, H], F8, tag="wqr")
                    w1ts.append(w1_t)
                    wqrts.append(wqr_t)

                # --- DMA emission in NEED order (one effective serial pipe
                # per DGE). SWDGE (gpsimd) carries the two tile-0 gates in
                # parallel with the HWDGE pipe.
                x_cuts = sorted(set(min(c, B) for c in [0, 512, 2048, 4096, B]))
                xsp = [c for c in zip(x_cuts[:-1], x_cuts[1:]) if c[1] > c[0]]
                nc.gpsimd.dma_start(xt[:, 0 : xsp[0][1]], xTd[:, 0 : xsp[0][1]])
                nc.gpsimd.dma_start(w1ts[0][:, :], w1d[0, :, :])
                nc.sync.dma_start(b2t[:, :], b2d[:, :])
                nc.sync.dma_start(wqrts[0][:, :, :], wqrd[0, :, :, :])
                nc.sync.dma_start(w3t[:, 0, :, :], w3d[:, 0, :, :])
                nc.sync.dma_start(w3t[:, 1:, :, :], w3d[:, 1:, :, :])
                xi = 1
                if xi < len(xsp):
                    c0, c1 = xsp[xi]
                    nc.sync.dma_start(xt[:, c0:c1], xTd[:, c0:c1])
                    xi += 1
                for g in range(1, P):
                    nc.sync.dma_start(w1ts[g][:, :], w1d[g, :, :])
                    nc.sync.dma_start(wqrts[g][:, :, :], wqrd[g, :, :, :])
                    if xi < len(xsp):
                        c0, c1 = xsp[xi]
                        nc.sync.dma_start(xt[:, c0:c1], xTd[:, c0:c1])
                        xi += 1

                # --- software-pipelined body:
                # stage A(t): L1 matmuls + H/L fp8 pair evictions
                # stage B(t): L2 DoubleRow matmuls + h2s eviction (Pool)
                # stage C(t): L3 token-major matmuls + y eviction + DMA out
                hts, lts, h2ts, stash = {}, {}, {}, {}

                def stage_a(t):
                    g, t0, tw = tiles[t]
                    gt = gpool.tile([128, 4, H], BF16, tag="gt")
                    ht = hlpool.tile([128, 4, H], F8, tag="ht")
                    lt = hlpool.tile([128, 4, H], F8, tag="lt")
                    for j in range(2):
                        ps = ps1pool.tile([128, 2, H], F32, tag="ps1")
                        for k in range(2):
                            nc.tensor.matmul(
                                ps[:, k, :tw],
                                w1ts[g][:, ts(2 * j + k, 128)],
                                xt[:, t0 : t0 + tw],
                                start=True,
                                stop=True,
                            )
                        # G = relu(x@W1 + b1), one 2-bank eviction (ACT)
                        nc.scalar.activation(
                            gt[:, 2 * j : 2 * j + 2, :tw], ps[:, :, :tw], RELU
                        )
                    for mp in range(4):
                        # H = fp8(G) on DVE (4x SBUF mode), L = G - H on Pool
                        nc.vector.tensor_scalar(
                            ht[:, mp, :tw], gt[:, mp, :tw], 0.0, None, ADD
                        )
                        nc.gpsimd.tensor_tensor(
                            lt[:, mp, :tw], gt[:, mp, :tw], ht[:, mp, :tw], SUB
                        )
                    hts[t], lts[t] = ht, lt

                def stage_b(t):
                    g, t0, tw = tiles[t]
                    ht, lt = hts.pop(t), lts.pop(t)
                    h2t = h2pool.tile([128, 4, H], BF16, tag="h2")
                    for mp in range(4):
                        ps = ps2pool.tile([128, H], F32, tag="ps2")
                        terms = [
                            (0, ht, 0), (2, ht, 0),   # H @ Wq
                            (0, lt, 0), (2, lt, 0),   # L @ Wq
                            (0, ht, 4), (2, ht, 4),   # H @ Wr
                        ]
                        for j, (c, mv, woff) in enumerate(terms):
                            nc.tensor.matmul(
                                ps[:, :tw],
                                wqrts[g][:, woff + c : woff + c + 2, ts(mp, 128)],
                                mv[:, c : c + 2, :tw],
                                start=(j == 0),
                                stop=(j == len(terms) - 1),
                                perf_mode=DR,
                            )
                        # h2s = relu(ps + 256*b2): mp0 on ACT, mp1-3 on DVE
                        bias_ap = b2t[:, 4 * g + mp : 4 * g + mp + 1]
                        if mp == 0:
                            nc.scalar.activation(
                                h2t[:, mp, :tw], ps[:, :tw], RELU, bias=bias_ap
                            )
                        else:
                            nc.vector.tensor_scalar(
                                h2t[:, mp, :tw], ps[:, :tw], bias_ap, 0.0,
                                ADD, MAX,
                            )
                    h2ts[t] = h2t

                def stage_c(t):
                    g, t0, tw = tiles[t]
                    h2t = h2ts.pop(t)
                    ntcf, szl = divmod(tw, 128)
                    ps = ps3pool.tile([128, 8, Z], F32, tag="ps3")
                    for tcb in range(ntcf + (1 if szl else 0)):
                        sz = 128 if tcb < ntcf else szl
                        for k in range(4):
                            nc.tensor.matmul(
                                ps[:sz, tcb, :],
                                h2t[:, k, tcb * 128 : tcb * 128 + sz],
                                w3t[:, g, k, :],
                                start=(k == 0),
                                stop=(k == 3),
                            )
                    yt = ypool.tile([128, 4, Z], F32, tag="y")
                    if ntcf:
                        nc.vector.tensor_scalar(
                            yt[:, :ntcf, :], ps[:, :ntcf, :], 0.0, None, ADD
                        )
                        nc.sync.dma_start(
                            yd[t, :, :ntcf, :], yt[:, :ntcf, :]
                        )
                    if szl:
                        nc.vector.tensor_scalar(
                            yt[:szl, ntcf, :], ps[:szl, ntcf, :], 0.0, None, ADD
                        )
                        nc.sync.dma_start(
                            yd[t, :szl, ntcf, :], yt[:szl, ntcf, :]
                        )

                for t in range(nt + 2):
                    if t < nt:
                        stage_a(t)
                    if 0 <= t - 1 < nt:
                        stage_b(t - 1)
                    if 0 <= t - 2 < nt:
                        stage_c(t - 2)

    nc.finalize()
    return nc


def _q8(x):
    return np.asarray(x, dtype=NP_F8)


def _pack_inputs(latents, actions, order, counts, pcounts, Bp, tiles,
                 W1, b1, W2, b2, W3, b3):
    """Per-core inputs. Core i handles inner MLP i for every row."""
    lat_s = latents[order]                       # [B, 512]
    act_s = actions[order]                       # [B, 8]
    spans = []                                   # (padded off, raw off, n)
    po = ro = 0
    for n, pn in zip(counts, pcounts):
        spans.append((po, ro, n))
        po += pn
        ro += n
    in_maps = []
    for i in range(NCORES):
        xT = np.zeros((INB, Bp), dtype=NP_BF16)
        for po, ro, n in spans:
            xT[:Z, po : po + n] = lat_s[ro : ro + n, i * Z : (i + 1) * Z].T
            xT[Z:IN, po : po + n] = act_s[ro : ro + n].T
            xT[IN, po : po + n] = 1.0

        w1 = np.zeros((P, INB, H), dtype=NP_BF16)
        w1[:, :IN, :] = W1[:, i]
        w1[:, IN, :] = b1[:, i]

        wqr = np.zeros((P, 128, 8, H), dtype=NP_F8)
        w2s = (WSCALE * W2[:, i]).astype(np.float32)       # [P, 512, 512]
        wq = _q8(w2s)
        wr = _q8(w2s - wq.astype(np.float32))
        wqr[:, :, 0:4, :] = wq.reshape(P, 4, 128, H).transpose(0, 2, 1, 3)
        wqr[:, :, 4:8, :] = wr.reshape(P, 4, 128, H).transpose(0, 2, 1, 3)

        w3 = np.zeros((128, P, 4, Z), dtype=NP_BF16)
        w3[:, :, :, :] = (
            (W3[:, i] / WSCALE).reshape(P, 4, 128, Z).transpose(2, 0, 1, 3)
        )

        # [128, P, 4] -> col index g*4 + mp holds 256*b2[g,i][mp*128+p]
        b2a = np.ascontiguousarray(
            (WSCALE * b2[:, i]).reshape(P, 4, 128).transpose(2, 0, 1)
        ).reshape(128, 4 * P).astype(np.float32)

        in_maps.append({"xT": xT, "w1": w1, "wqr": wqr, "w3": w3, "b2": b2a})
    return in_maps


def _prepare(latents, actions, policy_indices, W1, b1, W2, b2, W3, b3):
    latents = np.asarray(latents, dtype=np.float32)
    actions = np.asarray(actions, dtype=np.float32)
    idx = np.asarray(policy_indices).astype(np.int64)
    W1 = np.ascontiguousarray(np.asarray(W1, dtype=np.float32))
    W2 = np.ascontiguousarray(np.asarray(W2, dtype=np.float32))
    W3 = np.ascontiguousarray(np.asarray(W3, dtype=np.float32))
    b1 = np.asarray(b1, dtype=np.float32)
    b2 = np.asarray(b2, dtype=np.float32)
    b3 = np.asarray(b3, dtype=np.float32)

    order = np.argsort(idx, kind="stable")
    counts = np.bincount(idx, minlength=P).tolist()
    # pad each group to a multiple of 4 dead columns, skipped at scatter
    pcounts = [-(-n // 4) * 4 for n in counts]
    Bp = sum(pcounts)
    tiles = _group_tiles(pcounts)

    in_maps = _pack_inputs(
        latents, actions, order, counts, pcounts, Bp, tiles,
        W1, b1, W2, b2, W3, b3
    )
    nc = _build_program(pcounts, Bp, repeat=REPEAT)
    return nc, in_maps, order, counts, pcounts


def _scatter_out(results, order, counts, pcounts, B, b3):
    """results[i]['y']: [nt, 128, 4, Z] tile slots; token tc*128+p of tile j
    lives at [j, p, tc]."""
    tiles = _group_tiles(pcounts)
    Bp = sum(pcounts)
    out = np.empty((B, D), dtype=np.float32)
    keep = np.zeros(Bp, dtype=bool)
    gid = np.zeros(Bp, dtype=np.int64)
    po = 0
    for g, (n, pn) in enumerate(zip(counts, pcounts)):
        keep[po : po + n] = True
        gid[po : po + pn] = g
        po += pn
    b3 = np.asarray(b3, dtype=np.float32)
    for i in range(NCORES):
        yflat = np.empty((Bp, Z), dtype=np.float32)
        ya = results[i]["y"]                     # [nt, 128, 4, Z]
        for j, (g, t0, tw) in enumerate(tiles):
            slot = ya[j].transpose(1, 0, 2).reshape(512, Z)  # (tc,p) rows
            yflat[t0 : t0 + tw] = slot[:tw]
        yk = yflat[keep] + b3[gid[keep], i]      # [B, Z] sorted order
        out[order, i * Z : (i + 1) * Z] = yk
    return out


def run_timed(nc, in_maps, iters=20):
    """Execute the finalized Bass program on the 8 cores via PJRT, timing
    repeated dispatches of the prebuilt executable (min over iters).
    Returns (per-core results, list of wall times in seconds)."""
    import time

    import jax
    from jax.experimental.shard_map import shard_map
    from jax.sharding import Mesh, NamedSharding, PartitionSpec

    from concourse import bass2jax, mybir as _mybir
    from concourse.bass2jax import _bass_exec_p, partition_id_tensor

    bass2jax.install_neuronx_cc_hook()
    n_cores = len(in_maps)

    partition_name = nc.partition_id_tensor.name if nc.partition_id_tensor else None
    in_names, out_names, out_avals, zero_outs = [], [], [], []
    for alloc in nc.m.functions[0].allocations:
        if not isinstance(alloc, _mybir.MemoryLocationSet):
            continue
        name = alloc.memorylocations[0].name
        if alloc.kind == "ExternalInput":
            if name != partition_name:
                in_names.append(name)
        elif alloc.kind == "ExternalOutput":
            out_names.append(name)
            shape = tuple(alloc.tensor_shape)
            dtype = _mybir.dt.np(alloc.dtype)
            out_avals.append(jax.core.ShapedArray(shape, dtype))
            zero_outs.append(np.zeros(shape, dtype))
    n_params = len(in_names)
    n_outs = len(out_avals)
    all_in_names = list(in_names) + out_names + (
        [partition_name] if partition_name else []
    )

    def _body(*args):
        operands = list(args)
        if partition_name is not None:
            operands.append(partition_id_tensor())
        outs = _bass_exec_p.bind(
            *operands,
            out_avals=tuple(out_avals),
            in_names=tuple(all_in_names),
            out_names=tuple(out_names),
            lowering_input_output_aliases=(),
            sim_require_finite=True,
            sim_require_nnan=True,
            nc=nc,
        )
        return tuple(outs)

    devices = jax.devices()[:n_cores]
    mesh = Mesh(np.asarray(devices), ("core",))
    spec = PartitionSpec("core")
    in_specs = (spec,) * (n_params + n_outs)
    out_specs = (spec,) * n_outs
    donate = tuple(range(n_params, n_params + n_outs))
    sharded = jax.jit(
        shard_map(_body, mesh=mesh, in_specs=in_specs, out_specs=out_specs,
                  check_rep=False),
        donate_argnums=donate,
        keep_unused=True,
    )
    sh = NamedSharding(mesh, spec)
    concat_in = [
        jax.device_put(
            np.concatenate([np.asarray(in_maps[c][nm]) for c in range(n_cores)], 0),
            sh,
        )
        for nm in in_names
    ]

    def fresh_zeros():
        return [
            jax.device_put(
                np.zeros((n_cores * z.shape[0], *z.shape[1:]), z.dtype), sh
            )
            for z in zero_outs
        ]

    out_arrs = sharded(*concat_in, *fresh_zeros())  # warmup + result
    jax.block_until_ready(out_arrs)
    results = [
        {
            nm: np.asarray(out_arrs[i]).reshape(n_cores, *out_avals[i].shape)[c]
            for i, nm in enumerate(out_names)
        }
        for c in range(n_cores)
    ]

    staged = [fresh_zeros() for _ in range(iters)]
    jax.block_until_ready(staged)
    import jax.numpy as jnp

    reduce_fn = jax.jit(lambda a: jnp.sum(a[:, :4]))
    times = []
    for z in staged:
        t0 = time.perf_counter()
        o = sharded(*concat_in, *z)
        float(reduce_fn(o[0]))  # tiny dependent reduction forces completion
        times.append(time.perf_counter() - t0)
    return results, times


def kernel(latents, actions, policy_indices, W1, b1, W2, b2, W3, b3):
    global LAST_RESULT
    nc, in_maps, order, counts, pcounts = _prepare(
        latents, actions, policy_indices, W1, b1, W2, b2, W3, b3
    )
    res = run_bass_kernel_spmd(nc, in_maps, list(range(NCORES)), trace=TRACE)
    LAST_RESULT = res
    return _scatter_out(
        res.results, order, counts, pcounts, np.asarray(latents).shape[0],
        np.asarray(b3, dtype=np.float32),
    )


# revision 51
# speedup vs baseline: 1.2029x; 1.2029x over previous
"""Inner-policy-sharded Trainium2 kernel for DecoupledDynamicsModel (MoE).

Model: B=8192 rows; each row selects one of P=8 outer policies via
policy_indices; the selected policy runs 8 inner MLPs (72 -> 512 -> 512 -> 64)
on (latent chunk, action) and the 8 inner outputs concatenate to 512 dims.

Sharding: by INNER policy. Core i computes inner MLP i for every row, using
the row's outer-policy weight set W*[outer, i]. Rows are sorted by outer
policy on the host so tokens form 8 contiguous groups; within a group the
weights are stationary. Perfect load balance, no capacity padding.

Numerics / engine plan (vs the f32r baseline):
- Layer 1 (K=73 incl. a ones-row carrying b1): bf16 feature-major matmul,
  4 m-chunks. PSUM holds x@W1 + b1 directly.
- Layer 1 eviction produces the layer-2 moving operand as an fp8 PAIR:
    H = fp8(relu(ps))            on ACT (activation Relu -> float8e4)
    L = fp8(relu(ps) - H)        on DVE (one fused scalar_tensor_tensor)
  so h1 ~= H + L to ~0.1% while both factors are fp8.
- Layer 2: fp8e4 DoubleRow matmuls (0.5 cycles/row): 3 compensated terms
  H@Wq + L@Wq + H@Wr where Wq = fp8(256*W2), Wr = fp8(256*W2 - Wq) are
  split on the host. All three share one PSUM scale (256). 6 DoubleRow
  matmuls per m-chunk replace 16 full-rate k-chunks.
- Layer 2 eviction h2s = max(ps + 256*b2, 0) -> bf16 on Pool (gpsimd).
  h2s = 256*h2; the 1/256 is folded into W3 on the host.
- Layer 3: token-major bf16 matmul: stationary = h2s chunk [128, 128 tok],
  moving = W3/256 [128, 64] -> PSUM [tokens, 64]; 2 cycles/token instead
  of 4 (no half-empty 64-wide output partitions). b3 is added on the host.
- y eviction on ACT (Copy), DMA out token-major per tile slot.

PE work: 4 (L1) + 12 (L2) + 2 (L3) = 18 cycles/token vs 24 for the f32r
baseline. Emission is software-pipelined [L1(t), L2(t-1), L3(t-2)] so the
PE never waits on same-tile evictions.
"""

import sys

sys.path.insert(0, "/opt/trn_rl_repo")

import numpy as np
import ml_dtypes

import concourse.bass as bass
from concourse import bacc
import concourse.mybir as mybir
import concourse.tile as tile
from concourse.bass import ts
from concourse.bass_utils import run_bass_kernel_spmd

P = 8          # outer policies == n_cores == inner MLPs per policy
Z = 64         # per-policy latent dim
D = P * Z      # 512
A = 8          # action dim
IN = Z + A     # 72, MLP input dim
INB = IN + 1   # 73: ones-row carries b1 into the L1 matmul
H = 512        # hidden dim
NCORES = 8

F32 = mybir.dt.float32
BF16 = mybir.dt.bfloat16
F8 = mybir.dt.float8e4
RELU = mybir.ActivationFunctionType.Relu
COPY = mybir.ActivationFunctionType.Copy
ADD = mybir.AluOpType.add
MAX = mybir.AluOpType.max
SUB = mybir.AluOpType.subtract
DR = mybir.MatmulPerfMode.DoubleRow

NP_BF16 = ml_dtypes.bfloat16
NP_F8 = ml_dtypes.float8_e4m3

WSCALE = 256.0  # host scale on W2 before fp8 split; folded back via b2/W3

TRACE = False
REPEAT = 1
LAST_RESULT = None


def _group_tiles(counts):
    """Token tiles for the sorted stream: each tile stays inside one outer-
    policy group; sizes <=512, multiples of 4 (group padding). The final
    group ends on its smallest tile so the pipeline drain is short."""
    tiles = []
    off = 0
    for g, n in enumerate(counts):
        r = n
        gt = []
        if g == 0 and r > 768:
            # small leading tile so compute starts on fewer loaded bytes
            gt.append(256)
            r -= 256
        if g == len(counts) - 1:
            # full 512s then a small remainder tile to shorten the drain
            while r > 512:
                gt.append(512)
                r -= 512
            if r:
                gt.append(r)
        else:
            while r > 0:
                if r > 768:
                    t = 512
                elif r > 512:
                    t = r - 256
                else:
                    t = r
                gt.append(t)
                r -= t
        for t in gt:
            tiles.append((g, off, t))
            off += t
    return tiles


def _build_program(counts, B, repeat=1):
    tiles = _group_tiles(counts)
    nt = len(tiles)
    nc = bacc.Bacc()

    xTd = nc.declare_dram_parameter("xT", [INB, B], BF16, isOutput=False)
    w1d = nc.declare_dram_parameter("w1", [P, INB, H], BF16, isOutput=False)
    wqrd = nc.declare_dram_parameter("wqr", [P, 128, 8, H], F8, isOutput=False)
    w3d = nc.declare_dram_parameter("w3", [128, P, 4, Z], BF16, isOutput=False)
    b2d = nc.declare_dram_parameter("b2", [128, 4 * P], F32, isOutput=False)
    yd = nc.declare_dram_parameter("y", [Z, nt * 512], F32, isOutput=True)

    with tile.TileContext(nc) as tc:
        with (
            tc.tile_pool(name="w1p", bufs=P) as w1pool,
            tc.tile_pool(name="wqrp", bufs=P) as wqrpool,
            tc.tile_pool(name="cst", bufs=1) as cstpool,
            tc.tile_pool(name="xs", bufs=1) as xpool,
            tc.tile_pool(name="gs", bufs=3) as gpool,
            tc.tile_pool(name="hl", bufs=3) as hlpool,
            tc.tile_pool(name="h2", bufs=4) as h2pool,
            tc.tile_pool(name="ys", bufs=3) as ypool,
            tc.tile_pool(name="ps1", bufs=2, space="PSUM") as ps1pool,
            tc.tile_pool(name="ps2", bufs=4, space="PSUM") as ps2pool,
        ):
            for _rep in range(repeat):
                xt = xpool.tile([INB, B], BF16, tag="x")
                w3t = cstpool.tile([128, P, 4, Z], BF16, tag="w3")
                b2t = cstpool.tile([128, 4 * P], F32, tag="b2")
                w1ts, wqrts = [], []
                for _g in range(P):
                    w1_t = w1pool.tile([INB, H], BF16, tag="w1")
                    wqr_t = wqrpool.tile([128, 8, H], F8, tag="wqr")
                    w1ts.append(w1_t)
                    wqrts.append(wqr_t)

                # --- PE warmup scratch: memset FIRST on Pool so the warmup
                # matmuls are not queued behind SWDGE descriptor generation
                scr = cstpool.tile([128, 512], BF16, tag="scr")
                nc.gpsimd.memset(scr[:, 0:320], 0)

                # --- DMA emission in NEED order (one effective serial pipe
                # per DGE). SWDGE (gpsimd) carries the tile-0 x gate in
                # parallel with the HWDGE pipe.
                x_cuts = sorted(set(min(c, B) for c in [0, 256, 1024, 2048, 4096, B]))
                xsp = [c for c in zip(x_cuts[:-1], x_cuts[1:]) if c[1] > c[0]]
                nc.gpsimd.dma_start(xt[:, 0 : xsp[0][1]], xTd[:, 0 : xsp[0][1]])
                nc.sync.dma_start(w1ts[0][:, :], w1d[0, :, :])
                xi = 1
                if xi < len(xsp):
                    # x for tile 1 ahead of the bulky weight transfers
                    c0, c1 = xsp[xi]
                    nc.sync.dma_start(xt[:, c0:c1], xTd[:, c0:c1])
                    xi += 1
                nc.sync.dma_start(wqrts[0][:, :, :], wqrd[0, :, :, :])
                nc.sync.dma_start(b2t[:, :], b2d[:, :])
                nc.sync.dma_start(w3t[:, 0, :, :], w3d[:, 0, :, :])
                nc.sync.dma_start(w3t[:, 1:, :, :], w3d[:, 1:, :, :])
                for g in range(1, P):
                    nc.sync.dma_start(w1ts[g][:, :], w1d[g, :, :])
                    nc.sync.dma_start(wqrts[g][:, :, :], wqrd[g, :, :, :])
                    if xi < len(xsp):
                        c0, c1 = xsp[xi]
                        nc.sync.dma_start(xt[:, c0:c1], xTd[:, c0:c1])
                        xi += 1

                # --- PE warmup: dummy matmuls on the zeroed scratch tile keep
                # the PE busy through the initial DMA wait and start the
                # p-state ramp clock before real work arrives.
                wps = ps1pool.tile([128, 2, H], F32, tag="ps1")
                for j in range(6):
                    nc.tensor.matmul(
                        wps[:, 0, 0:320],
                        scr[0:73, 0:128],
                        scr[0:73, 0:320],
                        start=(j == 0),
                        stop=(j == 5),
                    )
                nc.vector.tensor_scalar(scr[0:1, 0:8].bitcast(F32),
                                        wps[0:1, 0, 0:4], 0.0, None, ADD)

                # --- software-pipelined body:
                # stage A(t): L1 matmuls + H/L fp8 pair evictions
                # stage B(t): L2 DoubleRow matmuls + h2s eviction
                # stage C(t): L3 feature-major matmuls + y eviction + DMA out
                hts, lts, h2ts, stash = {}, {}, {}, {}

                def stage_a(t):
                    g, t0, tw = tiles[t]
                    ht = hlpool.tile([128, 4, H], F8, tag="ht")
                    lt = hlpool.tile([128, 4, H], F8, tag="lt")
                    for j in range(2):
                        ps = ps1pool.tile([128, 2, H], F32, tag="ps1")
                        for k in range(2):
                            nc.tensor.matmul(
                                ps[:, k, :tw],
                                w1ts[g][:, ts(2 * j + k, 128)],
                                xt[:, t0 : t0 + tw],
                                start=True,
                                stop=True,
                            )
                        # h1 = relu(x@W1 + b1) straight to an fp8 pair:
                        # H = fp8(relu(ps)) on ACT, L = relu(ps) - H on DVE
                        # (both 2-bank PSUM reads; no bf16 intermediate)
                        nc.scalar.activation(
                            ht[:, 2 * j : 2 * j + 2, :tw], ps[:, :, :tw], RELU
                        )
                        nc.vector.scalar_tensor_tensor(
                            lt[:, 2 * j : 2 * j + 2, :tw], ps[:, :, :tw], 0.0,
                            ht[:, 2 * j : 2 * j + 2, :tw], MAX, SUB,
                        )
                    hts[t], lts[t] = ht, lt

                def stage_b(t):
                    g, t0, tw = tiles[t]
                    ht, lt = hts.pop(t), lts.pop(t)
                    h2t = h2pool.tile([128, 4, H], BF16, tag="h2")
                    for mp in range(4):
                        ps = ps2pool.tile([128, H], F32, tag="ps2")
                        terms = [
                            (0, ht, 0), (2, ht, 0),   # H @ Wq
                            (0, ht, 4), (2, ht, 4),   # H @ Wr
                            (0, lt, 0), (2, lt, 0),   # L @ Wq (L lands last)
                        ]
                        for j, (c, mv, woff) in enumerate(terms):
                            nc.tensor.matmul(
                                ps[:, :tw],
                                wqrts[g][:, woff + c : woff + c + 2, ts(mp, 128)],
                                mv[:, c : c + 2, :tw],
                                start=(j == 0),
                                stop=(j == len(terms) - 1),
                                perf_mode=DR,
                            )
                        # h2s = relu(ps + 256*b2); mp0-1 on ACT, mp2-3 on DVE
                        bias_ap = b2t[:, 4 * g + mp : 4 * g + mp + 1]
                        if mp < 2:
                            nc.scalar.activation(
                                h2t[:, mp, :tw], ps[:, :tw], RELU, bias=bias_ap
                            )
                        else:
                            nc.vector.tensor_scalar(
                                h2t[:, mp, :tw], ps[:, :tw], bias_ap, 0.0,
                                ADD, MAX,
                            )
                    h2ts[t] = h2t

                def stage_c(t):
                    g, t0, tw = tiles[t]
                    h2t = h2ts.pop(t)
                    # feature-major layer 3: out [64 feats, tw tokens];
                    # M=64 wastes half the PE cols but costs the same under
                    # the row-cost model and needs only 4 matmuls/tile.
                    # PSUM comes from the shared L2/L3 pool (uses rows 0-63).
                    ps = ps2pool.tile([128, 512], F32, tag="ps2")
                    for k in range(4):
                        nc.tensor.matmul(
                            ps[:Z, :tw],
                            w3t[:, g, k, :],
                            h2t[:, k, :tw],
                            start=(k == 0),
                            stop=(k == 3),
                        )
                    # final two tiles share one SBUF tile and one merged DMA
                    # to shorten the drain (needs tiles[nt-2] full-width)
                    merge = nt >= 2 and tiles[nt - 2][2] == 512
                    if merge and t == nt - 2:
                        ytp = ypool.tile([Z, 1024], F32, tag="yp")
                        nc.scalar.activation(ytp[:, 0:512], ps[:Z, :tw], COPY)
                        stash["ytp"] = ytp
                    elif merge and t == nt - 1:
                        ytp = stash.pop("ytp")
                        nc.scalar.activation(
                            ytp[:, 512 : 512 + tw], ps[:Z, :tw], COPY
                        )
                        nc.sync.dma_start(
                            yd[:, (nt - 2) * 512 : (nt - 2) * 512 + 512 + tw],
                            ytp[:, 0 : 512 + tw],
                        )
                    else:
                        yt = ypool.tile([Z, 512], F32, tag="y")
                        nc.scalar.activation(yt[:, :tw], ps[:Z, :tw], COPY)
                        nc.sync.dma_start(
                            yd[:, t * 512 : t * 512 + tw], yt[:, :tw]
                        )

                for t in range(nt + 2):
                    if t < nt:
                        stage_a(t)
                    if 0 <= t - 1 < nt:
                        stage_b(t - 1)
                    if 0 <= t - 2 < nt:
                        stage_c(t - 2)

    nc.finalize()
    return nc


def _q8(x):
    return np.asarray(x, dtype=NP_F8)


def _pack_inputs(latents, actions, order, counts, pcounts, Bp, tiles,
                 W1, b1, W2, b2, W3, b3):
    """Per-core inputs. Core i handles inner MLP i for every row."""
    lat_s = latents[order]                       # [B, 512]
    act_s = actions[order]                       # [B, 8]
    spans = []                                   # (padded off, raw off, n)
    po = ro = 0
    for n, pn in zip(counts, pcounts):
        spans.append((po, ro, n))
        po += pn
        ro += n
    in_maps = []
    for i in range(NCORES):
        xT = np.zeros((INB, Bp), dtype=NP_BF16)
        for po, ro, n in spans:
            xT[:Z, po : po + n] = lat_s[ro : ro + n, i * Z : (i + 1) * Z].T
            xT[Z:IN, po : po + n] = act_s[ro : ro + n].T
            xT[IN, po : po + n] = 1.0

        w1 = np.zeros((P, INB, H), dtype=NP_BF16)
        w1[:, :IN, :] = W1[:, i]
        w1[:, IN, :] = b1[:, i]

        wqr = np.zeros((P, 128, 8, H), dtype=NP_F8)
        w2s = (WSCALE * W2[:, i]).astype(np.float32)       # [P, 512, 512]
        wq = _q8(w2s)
        wr = _q8(w2s - wq.astype(np.float32))
        wqr[:, :, 0:4, :] = wq.reshape(P, 4, 128, H).transpose(0, 2, 1, 3)
        wqr[:, :, 4:8, :] = wr.reshape(P, 4, 128, H).transpose(0, 2, 1, 3)

        w3 = np.zeros((128, P, 4, Z), dtype=NP_BF16)
        w3[:, :, :, :] = (
            (W3[:, i] / WSCALE).reshape(P, 4, 128, Z).transpose(2, 0, 1, 3)
        )

        # [128, P, 4] -> col index g*4 + mp holds 256*b2[g,i][mp*128+p]
        b2a = np.ascontiguousarray(
            (WSCALE * b2[:, i]).reshape(P, 4, 128).transpose(2, 0, 1)
        ).reshape(128, 4 * P).astype(np.float32)

        in_maps.append({"xT": xT, "w1": w1, "wqr": wqr, "w3": w3, "b2": b2a})
    return in_maps


def _prepare(latents, actions, policy_indices, W1, b1, W2, b2, W3, b3):
    latents = np.asarray(latents, dtype=np.float32)
    actions = np.asarray(actions, dtype=np.float32)
    idx = np.asarray(policy_indices).astype(np.int64)
    W1 = np.ascontiguousarray(np.asarray(W1, dtype=np.float32))
    W2 = np.ascontiguousarray(np.asarray(W2, dtype=np.float32))
    W3 = np.ascontiguousarray(np.asarray(W3, dtype=np.float32))
    b1 = np.asarray(b1, dtype=np.float32)
    b2 = np.asarray(b2, dtype=np.float32)
    b3 = np.asarray(b3, dtype=np.float32)

    order = np.argsort(idx, kind="stable")
    counts = np.bincount(idx, minlength=P).tolist()
    # pad each group to a multiple of 4 dead columns, skipped at scatter
    pcounts = [-(-n // 4) * 4 for n in counts]
    Bp = sum(pcounts)
    tiles = _group_tiles(pcounts)

    in_maps = _pack_inputs(
        latents, actions, order, counts, pcounts, Bp, tiles,
        W1, b1, W2, b2, W3, b3
    )
    nc = _build_program(pcounts, Bp, repeat=REPEAT)
    return nc, in_maps, order, counts, pcounts


def _scatter_out(results, order, counts, pcounts, B, b3):
    """results[i]['y']: [nt, Z, 512] feature-major tile slots."""
    tiles = _group_tiles(pcounts)
    Bp = sum(pcounts)
    out = np.empty((B, D), dtype=np.float32)
    keep = np.zeros(Bp, dtype=bool)
    gid = np.zeros(Bp, dtype=np.int64)
    po = 0
    for g, (n, pn) in enumerate(zip(counts, pcounts)):
        keep[po : po + n] = True
        gid[po : po + pn] = g
        po += pn
    b3 = np.asarray(b3, dtype=np.float32)
    for i in range(NCORES):
        yflat = np.empty((Bp, Z), dtype=np.float32)
        ya = results[i]["y"]                     # [Z, nt*512]
        for j, (g, t0, tw) in enumerate(tiles):
            yflat[t0 : t0 + tw] = ya[:, j * 512 : j * 512 + tw].T
        yk = yflat[keep] + b3[gid[keep], i]      # [B, Z] sorted order
        out[order, i * Z : (i + 1) * Z] = yk
    return out


def run_timed(nc, in_maps, iters=20):
    """Execute the finalized Bass program on the 8 cores via PJRT, timing
    repeated dispatches of the prebuilt executable (min over iters).
    Returns (per-core results, list of wall times in seconds)."""
    import time

    import jax
    from jax.experimental.shard_map import shard_map
    from jax.sharding import Mesh, NamedSharding, PartitionSpec

    from concourse import bass2jax, mybir as _mybir
    from concourse.bass2jax import _bass_exec_p, partition_id_tensor

    bass2jax.install_neuronx_cc_hook()
    n_cores = len(in_maps)

    partition_name = nc.partition_id_tensor.name if nc.partition_id_tensor else None
    in_names, out_names, out_avals, zero_outs = [], [], [], []
    for alloc in nc.m.functions[0].allocations:
        if not isinstance(alloc, _mybir.MemoryLocationSet):
            continue
        name = alloc.memorylocations[0].name
        if alloc.kind == "ExternalInput":
            if name != partition_name:
                in_names.append(name)
        elif alloc.kind == "ExternalOutput":
            out_names.append(name)
            shape = tuple(alloc.tensor_shape)
            dtype = _mybir.dt.np(alloc.dtype)
            out_avals.append(jax.core.ShapedArray(shape, dtype))
            zero_outs.append(np.zeros(shape, dtype))
    n_params = len(in_names)
    n_outs = len(out_avals)
    all_in_names = list(in_names) + out_names + (
        [partition_name] if partition_name else []
    )

    def _body(*args):
        operands = list(args)
        if partition_name is not None:
            operands.append(partition_id_tensor())
        outs = _bass_exec_p.bind(
            *operands,
            out_avals=tuple(out_avals),
            in_names=tuple(all_in_names),
            out_names=tuple(out_names),
            lowering_input_output_aliases=(),
            sim_require_finite=True,
            sim_require_nnan=True,
            nc=nc,
        )
        return tuple(outs)

    devices = jax.devices()[:n_cores]
    mesh = Mesh(np.asarray(devices), ("core",))
    spec = PartitionSpec("core")
    in_specs = (spec,) * (n_params + n_outs)
    out_specs = (spec,) * n_outs
    donate = tuple(range(n_params, n_params + n_outs))
    sharded = jax.jit(
        shard_map(_body, mesh=mesh, in_specs=in_specs, out_specs=out_specs,
                  check_rep=False),
        donate_argnums=donate,
        keep_unused=True,
    )
    sh = NamedSharding(mesh, spec)
    concat_in = [
        jax.device_put(
            np.concatenate([np.asarray(in_maps[c][nm]) for c in range(n_cores)], 0),
            sh,
        )
        for nm in in_names
    ]

    def fresh_zeros():
        return [
            jax.device_put(
                np.zeros((n_cores * z.shape[0], *z.shape[1:]), z.dtype), sh
            )
            for z in zero_outs
        ]

    out_arrs = sharded(*concat_in, *fresh_zeros())  # warmup + result
    jax.block_until_ready(out_arrs)
    results = [
        {
            nm: np.asarray(out_arrs[i]).reshape(n_cores, *out_avals[i].shape)[c]
            for i, nm in enumerate(out_names)
        }
        for c in range(n_cores)
    ]

    staged = [fresh_zeros() for _ in range(iters)]
    jax.block_until_ready(staged)
    import jax.numpy as jnp

    reduce_fn = jax.jit(lambda a: jnp.sum(a[:, :4]))
    times = []
    for z in staged:
        t0 = time.perf_counter()
        o = sharded(*concat_in, *z)
        float(reduce_fn(o[0]))  # tiny dependent reduction forces completion
        times.append(time.perf_counter() - t0)
    return results, times


def kernel(latents, actions, policy_indices, W1, b1, W2, b2, W3, b3):
    global LAST_RESULT
    nc, in_maps, order, counts, pcounts = _prepare(
        latents, actions, policy_indices, W1, b1, W2, b2, W3, b3
    )
    res = run_bass_kernel_spmd(nc, in_maps, list(range(NCORES)), trace=TRACE)
    LAST_RESULT = res
    return _scatter_out(
        res.results, order, counts, pcounts, np.asarray(latents).shape[0],
        np.asarray(b3, dtype=np.float32),
    )


# revision 52
# speedup vs baseline: 1.2089x; 1.0049x over previous
"""Inner-policy-sharded Trainium2 kernel for DecoupledDynamicsModel (MoE).

Model: B=8192 rows; each row selects one of P=8 outer policies via
policy_indices; the selected policy runs 8 inner MLPs (72 -> 512 -> 512 -> 64)
on (latent chunk, action) and the 8 inner outputs concatenate to 512 dims.

Sharding: by INNER policy. Core i computes inner MLP i for every row, using
the row's outer-policy weight set W*[outer, i]. Rows are sorted by outer
policy on the host so tokens form 8 contiguous groups; within a group the
weights are stationary. Perfect load balance, no capacity padding.

Numerics / engine plan (vs the f32r baseline):
- Layer 1 (K=73 incl. a ones-row carrying b1): bf16 feature-major matmul,
  4 m-chunks. PSUM holds x@W1 + b1 directly.
- Layer 1 eviction produces the layer-2 moving operand as an fp8 PAIR:
    H = fp8(relu(ps))            on ACT (activation Relu -> float8e4)
    L = fp8(relu(ps) - H)        on DVE (one fused scalar_tensor_tensor)
  so h1 ~= H + L to ~0.1% while both factors are fp8.
- Layer 2: fp8e4 DoubleRow matmuls (0.5 cycles/row): 3 compensated terms
  H@Wq + L@Wq + H@Wr where Wq = fp8(256*W2), Wr = fp8(256*W2 - Wq) are
  split on the host. All three share one PSUM scale (256). 6 DoubleRow
  matmuls per m-chunk replace 16 full-rate k-chunks.
- Layer 2 eviction h2s = max(ps + 256*b2, 0) -> bf16 on Pool (gpsimd).
  h2s = 256*h2; the 1/256 is folded into W3 on the host.
- Layer 3: token-major bf16 matmul: stationary = h2s chunk [128, 128 tok],
  moving = W3/256 [128, 64] -> PSUM [tokens, 64]; 2 cycles/token instead
  of 4 (no half-empty 64-wide output partitions). b3 is added on the host.
- y eviction on ACT (Copy), DMA out token-major per tile slot.

PE work: 4 (L1) + 12 (L2) + 2 (L3) = 18 cycles/token vs 24 for the f32r
baseline. Emission is software-pipelined [L1(t), L2(t-1), L3(t-2)] so the
PE never waits on same-tile evictions.
"""

import sys

sys.path.insert(0, "/opt/trn_rl_repo")

import numpy as np
import ml_dtypes

import concourse.bass as bass
from concourse import bacc
import concourse.mybir as mybir
import concourse.tile as tile
from concourse.bass import ts
from concourse.bass_utils import run_bass_kernel_spmd

P = 8          # outer policies == n_cores == inner MLPs per policy
Z = 64         # per-policy latent dim
D = P * Z      # 512
A = 8          # action dim
IN = Z + A     # 72, MLP input dim
INB = IN + 1   # 73: ones-row carries b1 into the L1 matmul
H = 512        # hidden dim
NCORES = 8

F32 = mybir.dt.float32
BF16 = mybir.dt.bfloat16
F8 = mybir.dt.float8e4
RELU = mybir.ActivationFunctionType.Relu
COPY = mybir.ActivationFunctionType.Copy
ADD = mybir.AluOpType.add
MAX = mybir.AluOpType.max
SUB = mybir.AluOpType.subtract
DR = mybir.MatmulPerfMode.DoubleRow

NP_BF16 = ml_dtypes.bfloat16
NP_F8 = ml_dtypes.float8_e4m3

WSCALE = 256.0  # host scale on W2 before fp8 split; folded back via b2/W3

TRACE = False
REPEAT = 1
LAST_RESULT = None


def _group_tiles(counts):
    """Token tiles for the sorted stream: each tile stays inside one outer-
    policy group; sizes <=512, multiples of 4 (group padding). The final
    group ends on its smallest tile so the pipeline drain is short."""
    tiles = []
    off = 0
    for g, n in enumerate(counts):
        r = n
        gt = []
        if g == 0 and r > 768:
            # small leading tile so compute starts on fewer loaded bytes
            gt.append(256)
            r -= 256
        if g == len(counts) - 1:
            # full 512s then a small remainder tile to shorten the drain
            while r > 512:
                gt.append(512)
                r -= 512
            if r:
                gt.append(r)
        else:
            while r > 0:
                if r > 768:
                    t = 512
                elif r > 512:
                    t = r - 256
                else:
                    t = r
                gt.append(t)
                r -= t
        for t in gt:
            tiles.append((g, off, t))
            off += t
    return tiles


def _build_program(counts, B, repeat=1):
    tiles = _group_tiles(counts)
    nt = len(tiles)
    nc = bacc.Bacc()

    xTd = nc.declare_dram_parameter("xT", [INB, B], BF16, isOutput=False)
    w1d = nc.declare_dram_parameter("w1", [P, INB, H], BF16, isOutput=False)
    wqrd = nc.declare_dram_parameter("wqr", [P, 128, 8, H], F8, isOutput=False)
    w3d = nc.declare_dram_parameter("w3", [128, P, 4, Z], BF16, isOutput=False)
    b2d = nc.declare_dram_parameter("b2", [128, 4 * P], F32, isOutput=False)
    yd = nc.declare_dram_parameter("y", [Z, nt * 512], F32, isOutput=True)

    with tile.TileContext(nc) as tc:
        with (
            tc.tile_pool(name="w1p", bufs=P) as w1pool,
            tc.tile_pool(name="wqrp", bufs=P) as wqrpool,
            tc.tile_pool(name="cst", bufs=1) as cstpool,
            tc.tile_pool(name="xs", bufs=1) as xpool,
            tc.tile_pool(name="gs", bufs=3) as gpool,
            tc.tile_pool(name="hl", bufs=3) as hlpool,
            tc.tile_pool(name="h2", bufs=4) as h2pool,
            tc.tile_pool(name="ys", bufs=3) as ypool,
            tc.tile_pool(name="ps1", bufs=2, space="PSUM") as ps1pool,
            tc.tile_pool(name="ps2", bufs=4, space="PSUM") as ps2pool,
        ):
            for _rep in range(repeat):
                xt = xpool.tile([INB, B], BF16, tag="x")
                w3t = cstpool.tile([128, P, 4, Z], BF16, tag="w3")
                b2t = cstpool.tile([128, 4 * P], F32, tag="b2")
                w1ts, wqrts = [], []
                for _g in range(P):
                    w1_t = w1pool.tile([INB, H], BF16, tag="w1")
                    wqr_t = wqrpool.tile([128, 8, H], F8, tag="wqr")
                    w1ts.append(w1_t)
                    wqrts.append(wqr_t)

                # --- PE warmup scratch: memset FIRST on Pool so the warmup
                # matmuls are not queued behind SWDGE descriptor generation
                scr = cstpool.tile([128, 512], BF16, tag="scr")
                nc.gpsimd.memset(scr[:, 0:320], 0)

                # --- DMA emission in NEED order (one effective serial pipe
                # per DGE). SWDGE (gpsimd) carries the tile-0 x gate in
                # parallel with the HWDGE pipe.
                x_cuts = sorted(set(min(c, B) for c in [0, 256, 1024, 2048, 4096, B]))
                xsp = [c for c in zip(x_cuts[:-1], x_cuts[1:]) if c[1] > c[0]]
                nc.gpsimd.dma_start(xt[:, 0 : xsp[0][1]], xTd[:, 0 : xsp[0][1]])
                nc.sync.dma_start(w1ts[0][:, :], w1d[0, :, :])
                xi = 1
                if xi < len(xsp):
                    # x for tile 1 ahead of the bulky weight transfers
                    c0, c1 = xsp[xi]
                    nc.sync.dma_start(xt[:, c0:c1], xTd[:, c0:c1])
                    xi += 1
                nc.sync.dma_start(wqrts[0][:, :, :], wqrd[0, :, :, :])
                nc.sync.dma_start(b2t[:, :], b2d[:, :])
                nc.sync.dma_start(w3t[:, 0, :, :], w3d[:, 0, :, :])
                nc.sync.dma_start(w3t[:, 1:, :, :], w3d[:, 1:, :, :])
                while xi < len(xsp):
                    # bulk x rides SWDGE: Pool is otherwise idle and the
                    # HWDGE pipe stays clear for weights
                    c0, c1 = xsp[xi]
                    nc.gpsimd.dma_start(xt[:, c0:c1], xTd[:, c0:c1])
                    xi += 1
                for g in range(1, P):
                    nc.sync.dma_start(w1ts[g][:, :], w1d[g, :, :])
                    nc.sync.dma_start(wqrts[g][:, :, :], wqrd[g, :, :, :])

                # --- PE warmup: dummy matmuls on the zeroed scratch tile keep
                # the PE busy through the initial DMA wait and start the
                # p-state ramp clock before real work arrives.
                wps = ps1pool.tile([128, 2, H], F32, tag="ps1")
                for j in range(6):
                    nc.tensor.matmul(
                        wps[:, 0, 0:320],
                        scr[0:73, 0:128],
                        scr[0:73, 0:320],
                        start=(j == 0),
                        stop=(j == 5),
                    )
                nc.vector.tensor_scalar(scr[0:1, 0:8].bitcast(F32),
                                        wps[0:1, 0, 0:4], 0.0, None, ADD)

                # --- software-pipelined body:
                # stage A(t): L1 matmuls + H/L fp8 pair evictions
                # stage B(t): L2 DoubleRow matmuls + h2s eviction
                # stage C(t): L3 feature-major matmuls + y eviction + DMA out
                hts, lts, h2ts, stash = {}, {}, {}, {}

                def stage_a(t):
                    g, t0, tw = tiles[t]
                    ht = hlpool.tile([128, 4, H], F8, tag="ht")
                    lt = hlpool.tile([128, 4, H], F8, tag="lt")
                    for j in range(2):
                        ps = ps1pool.tile([128, 2, H], F32, tag="ps1")
                        for k in range(2):
                            nc.tensor.matmul(
                                ps[:, k, :tw],
                                w1ts[g][:, ts(2 * j + k, 128)],
                                xt[:, t0 : t0 + tw],
                                start=True,
                                stop=True,
                            )
                        # h1 = relu(x@W1 + b1) straight to an fp8 pair:
                        # H = fp8(relu(ps)) on ACT, L = relu(ps) - H on DVE
                        # (both 2-bank PSUM reads; no bf16 intermediate)
                        nc.scalar.activation(
                            ht[:, 2 * j : 2 * j + 2, :tw], ps[:, :, :tw], RELU
                        )
                        nc.vector.scalar_tensor_tensor(
                            lt[:, 2 * j : 2 * j + 2, :tw], ps[:, :, :tw], 0.0,
                            ht[:, 2 * j : 2 * j + 2, :tw], MAX, SUB,
                        )
                    hts[t], lts[t] = ht, lt

                def stage_b(t):
                    g, t0, tw = tiles[t]
                    ht, lt = hts.pop(t), lts.pop(t)
                    h2t = h2pool.tile([128, 4, H], BF16, tag="h2")
                    for mp in range(4):
                        ps = ps2pool.tile([128, H], F32, tag="ps2")
                        terms = [
                            (0, ht, 0), (2, ht, 0),   # H @ Wq
                            (0, ht, 4), (2, ht, 4),   # H @ Wr
                            (0, lt, 0), (2, lt, 0),   # L @ Wq (L lands last)
                        ]
                        for j, (c, mv, woff) in enumerate(terms):
                            nc.tensor.matmul(
                                ps[:, :tw],
                                wqrts[g][:, woff + c : woff + c + 2, ts(mp, 128)],
                                mv[:, c : c + 2, :tw],
                                start=(j == 0),
                                stop=(j == len(terms) - 1),
                                perf_mode=DR,
                            )
                        # h2s = relu(ps + 256*b2); mp0-1 on ACT, mp2-3 on DVE
                        bias_ap = b2t[:, 4 * g + mp : 4 * g + mp + 1]
                        if mp < 2:
                            nc.scalar.activation(
                                h2t[:, mp, :tw], ps[:, :tw], RELU, bias=bias_ap
                            )
                        else:
                            nc.vector.tensor_scalar(
                                h2t[:, mp, :tw], ps[:, :tw], bias_ap, 0.0,
                                ADD, MAX,
                            )
                    h2ts[t] = h2t

                def stage_c(t):
                    g, t0, tw = tiles[t]
                    h2t = h2ts.pop(t)
                    # feature-major layer 3: out [64 feats, tw tokens];
                    # M=64 wastes half the PE cols but costs the same under
                    # the row-cost model and needs only 4 matmuls/tile.
                    # PSUM comes from the shared L2/L3 pool (uses rows 0-63).
                    ps = ps2pool.tile([128, 512], F32, tag="ps2")
                    for k in range(4):
                        nc.tensor.matmul(
                            ps[:Z, :tw],
                            w3t[:, g, k, :],
                            h2t[:, k, :tw],
                            start=(k == 0),
                            stop=(k == 3),
                        )
                    # final two tiles share one SBUF tile and one merged DMA
                    # to shorten the drain (needs tiles[nt-2] full-width)
                    merge = nt >= 2 and tiles[nt - 2][2] == 512
                    if merge and t == nt - 2:
                        ytp = ypool.tile([Z, 1024], F32, tag="yp")
                        nc.scalar.activation(ytp[:, 0:512], ps[:Z, :tw], COPY)
                        stash["ytp"] = ytp
                    elif merge and t == nt - 1:
                        ytp = stash.pop("ytp")
                        nc.scalar.activation(
                            ytp[:, 512 : 512 + tw], ps[:Z, :tw], COPY
                        )
                        nc.sync.dma_start(
                            yd[:, (nt - 2) * 512 : (nt - 2) * 512 + 512 + tw],
                            ytp[:, 0 : 512 + tw],
                        )
                    else:
                        yt = ypool.tile([Z, 512], F32, tag="y")
                        nc.scalar.activation(yt[:, :tw], ps[:Z, :tw], COPY)
                        nc.sync.dma_start(
                            yd[:, t * 512 : t * 512 + tw], yt[:, :tw]
                        )

                for t in range(nt + 2):
                    if t < nt:
                        stage_a(t)
                    if 0 <= t - 1 < nt:
                        stage_b(t - 1)
                    if 0 <= t - 2 < nt:
                        stage_c(t - 2)

    nc.finalize()
    return nc


def _q8(x):
    return np.asarray(x, dtype=NP_F8)


def _pack_inputs(latents, actions, order, counts, pcounts, Bp, tiles,
                 W1, b1, W2, b2, W3, b3):
    """Per-core inputs. Core i handles inner MLP i for every row."""
    lat_s = latents[order]                       # [B, 512]
    act_s = actions[order]                       # [B, 8]
    spans = []                                   # (padded off, raw off, n)
    po = ro = 0
    for n, pn in zip(counts, pcounts):
        spans.append((po, ro, n))
        po += pn
        ro += n
    in_maps = []
    for i in range(NCORES):
        xT = np.zeros((INB, Bp), dtype=NP_BF16)
        for po, ro, n in spans:
            xT[:Z, po : po + n] = lat_s[ro : ro + n, i * Z : (i + 1) * Z].T
            xT[Z:IN, po : po + n] = act_s[ro : ro + n].T
            xT[IN, po : po + n] = 1.0

        w1 = np.zeros((P, INB, H), dtype=NP_BF16)
        w1[:, :IN, :] = W1[:, i]
        w1[:, IN, :] = b1[:, i]

        wqr = np.zeros((P, 128, 8, H), dtype=NP_F8)
        w2s = (WSCALE * W2[:, i]).astype(np.float32)       # [P, 512, 512]
        wq = _q8(w2s)
        wr = _q8(w2s - wq.astype(np.float32))
        wqr[:, :, 0:4, :] = wq.reshape(P, 4, 128, H).transpose(0, 2, 1, 3)
        wqr[:, :, 4:8, :] = wr.reshape(P, 4, 128, H).transpose(0, 2, 1, 3)

        w3 = np.zeros((128, P, 4, Z), dtype=NP_BF16)
        w3[:, :, :, :] = (
            (W3[:, i] / WSCALE).reshape(P, 4, 128, Z).transpose(2, 0, 1, 3)
        )

        # [128, P, 4] -> col index g*4 + mp holds 256*b2[g,i][mp*128+p]
        b2a = np.ascontiguousarray(
            (WSCALE * b2[:, i]).reshape(P, 4, 128).transpose(2, 0, 1)
        ).reshape(128, 4 * P).astype(np.float32)

        in_maps.append({"xT": xT, "w1": w1, "wqr": wqr, "w3": w3, "b2": b2a})
    return in_maps


def _prepare(latents, actions, policy_indices, W1, b1, W2, b2, W3, b3):
    latents = np.asarray(latents, dtype=np.float32)
    actions = np.asarray(actions, dtype=np.float32)
    idx = np.asarray(policy_indices).astype(np.int64)
    W1 = np.ascontiguousarray(np.asarray(W1, dtype=np.float32))
    W2 = np.ascontiguousarray(np.asarray(W2, dtype=np.float32))
    W3 = np.ascontiguousarray(np.asarray(W3, dtype=np.float32))
    b1 = np.asarray(b1, dtype=np.float32)
    b2 = np.asarray(b2, dtype=np.float32)
    b3 = np.asarray(b3, dtype=np.float32)

    order = np.argsort(idx, kind="stable")
    counts = np.bincount(idx, minlength=P).tolist()
    # pad each group to a multiple of 4 dead columns, skipped at scatter
    pcounts = [-(-n // 4) * 4 for n in counts]
    Bp = sum(pcounts)
    tiles = _group_tiles(pcounts)

    in_maps = _pack_inputs(
        latents, actions, order, counts, pcounts, Bp, tiles,
        W1, b1, W2, b2, W3, b3
    )
    nc = _build_program(pcounts, Bp, repeat=REPEAT)
    return nc, in_maps, order, counts, pcounts


def _scatter_out(results, order, counts, pcounts, B, b3):
    """results[i]['y']: [nt, Z, 512] feature-major tile slots."""
    tiles = _group_tiles(pcounts)
    Bp = sum(pcounts)
    out = np.empty((B, D), dtype=np.float32)
    keep = np.zeros(Bp, dtype=bool)
    gid = np.zeros(Bp, dtype=np.int64)
    po = 0
    for g, (n, pn) in enumerate(zip(counts, pcounts)):
        keep[po : po + n] = True
        gid[po : po + pn] = g
        po += pn
    b3 = np.asarray(b3, dtype=np.float32)
    for i in range(NCORES):
        yflat = np.empty((Bp, Z), dtype=np.float32)
        ya = results[i]["y"]                     # [Z, nt*512]
        for j, (g, t0, tw) in enumerate(tiles):
            yflat[t0 : t0 + tw] = ya[:, j * 512 : j * 512 + tw].T
        yk = yflat[keep] + b3[gid[keep], i]      # [B, Z] sorted order
        out[order, i * Z : (i + 1) * Z] = yk
    return out


def run_timed(nc, in_maps, iters=20):
    """Execute the finalized Bass program on the 8 cores via PJRT, timing
    repeated dispatches of the prebuilt executable (min over iters).
    Returns (per-core results, list of wall times in seconds)."""
    import time

    import jax
    from jax.experimental.shard_map import shard_map
    from jax.sharding import Mesh, NamedSharding, PartitionSpec

    from concourse import bass2jax, mybir as _mybir
    from concourse.bass2jax import _bass_exec_p, partition_id_tensor

    bass2jax.install_neuronx_cc_hook()
    n_cores = len(in_maps)

    partition_name = nc.partition_id_tensor.name if nc.partition_id_tensor else None
    in_names, out_names, out_avals, zero_outs = [], [], [], []
    for alloc in nc.m.functions[0].allocations:
        if not isinstance(alloc, _mybir.MemoryLocationSet):
            continue
        name = alloc.memorylocations[0].name
        if alloc.kind == "ExternalInput":
            if name != partition_name:
                in_names.append(name)
        elif alloc.kind == "ExternalOutput":
            out_names.append(name)
            shape = tuple(alloc.tensor_shape)
            dtype = _mybir.dt.np(alloc.dtype)
            out_avals.append(jax.core.ShapedArray(shape, dtype))
            zero_outs.append(np.zeros(shape, dtype))
    n_params = len(in_names)
    n_outs = len(out_avals)
    all_in_names = list(in_names) + out_names + (
        [partition_name] if partition_name else []
    )

    def _body(*args):
        operands = list(args)
        if partition_name is not None:
            operands.append(partition_id_tensor())
        outs = _bass_exec_p.bind(
            *operands,
            out_avals=tuple(out_avals),
            in_names=tuple(all_in_names),
            out_names=tuple(out_names),
            lowering_input_output_aliases=(),
            sim_require_finite=True,
            sim_require_nnan=True,
            nc=nc,
        )
        return tuple(outs)

    devices = jax.devices()[:n_cores]
    mesh = Mesh(np.asarray(devices), ("core",))
    spec = PartitionSpec("core")
    in_specs = (spec,) * (n_params + n_outs)
    out_specs = (spec,) * n_outs
    donate = tuple(range(n_params, n_params + n_outs))
    sharded = jax.jit(
        shard_map(_body, mesh=mesh, in_specs=in_specs, out_specs=out_specs,
                  check_rep=False),
        donate_argnums=donate,
        keep_unused=True,
    )
    sh = NamedSharding(mesh, spec)
    concat_in = [
        jax.device_put(
            np.concatenate([np.asarray(in_maps[c][nm]) for c in range(n_cores)], 0),
            sh,
        )
        for nm in in_names
    ]

    def fresh_zeros():
        return [
            jax.device_put(
                np.zeros((n_cores * z.shape[0], *z.shape[1:]), z.dtype), sh
            )
            for z in zero_outs
        ]

    out_arrs = sharded(*concat_in, *fresh_zeros())  # warmup + result
    jax.block_until_ready(out_arrs)
    results = [
        {
            nm: np.asarray(out_arrs[i]).reshape(n_cores, *out_avals[i].shape)[c]
            for i, nm in enumerate(out_names)
        }
        for c in range(n_cores)
    ]

    staged = [fresh_zeros() for _ in range(iters)]
    jax.block_until_ready(staged)
    import jax.numpy as jnp

    reduce_fn = jax.jit(lambda a: jnp.sum(a[:, :4]))
    times = []
    for z in staged:
        t0 = time.perf_counter()
        o = sharded(*concat_in, *z)
        float(reduce_fn(o[0]))  # tiny dependent reduction forces completion
        times.append(time.perf_counter() - t0)
    return results, times


def kernel(latents, actions, policy_indices, W1, b1, W2, b2, W3, b3):
    global LAST_RESULT
    nc, in_maps, order, counts, pcounts = _prepare(
        latents, actions, policy_indices, W1, b1, W2, b2, W3, b3
    )
    res = run_bass_kernel_spmd(nc, in_maps, list(range(NCORES)), trace=TRACE)
    LAST_RESULT = res
    return _scatter_out(
        res.results, order, counts, pcounts, np.asarray(latents).shape[0],
        np.asarray(b3, dtype=np.float32),
    )
